# revision 8
# baseline (speedup 1.0000x reference)
"""NestedAttention Trainium2 kernel.

Reference computation (per batch b):
  q_i = wq[i] @ x ; k_j = wk[j] @ x ; v_j = wv[j] @ x        (1x1 convs, r=64)
  for i: acc_i = sum_j softmax_m(q_i^T k_j / sqrt(r)) applied to v_j
  out = wo @ concat_i(acc_i) ; y = x * sigmoid(out)

Sharding: 8 cores = batch(4) x query-column-halves(2). Each core holds full
k/v (m = 2304 keys) and a 1152-wide slice of query columns n; no cross-core
communication is needed (softmax is over m, fully on-core).

Per-core dataflow ([m, n] logit layout so the softmax sum over m comes out of
the second matmul for free):
  T_ij[m, n]   = k_j^T q_i           (PE, bf16 inputs, K=r padded to 128)
  E_ij         = exp(T * 0.125)      (ScalarE, PSUM->SBUF bf16)
  P_aug[128,n] = [vT_j | ones64]^T @ E_ij   rows 0:64 = v@P, rows 64:128 = Z
  acc_i[r, n] += P_aug[0:64] * (1/Z)
  out[c, n]    = sum_i woT_i^T @ acc_i ; y = xn * sigmoid(out)
"""

import os
import numpy as np

B, C, H, W = 4, 256, 48, 48
N = H * W            # 2304 keys (m) per image
NSLICE = N // 2      # 1152 query columns (n) per core
R = 64               # reduced channels
P = 128
MT = N // P          # 18 m-tiles
KT = C // P          # 2 contraction tiles over channels
CHUNKS = [(0, 512), (512, 512), (1024, 128)]  # n chunks, PSUM-bank aligned
N_CORES = 8

NORM_MODE = os.environ.get("NESTED_NORM_MODE", "xquad")   # xquad | dmabcast
VT_LAYOUT = os.environ.get("NESTED_VT_LAYOUT", "interleave")  # 2dap | interleave

_CACHE = {}
LAST_RESULTS = None


def _build_program():
    from contextlib import ExitStack

    import concourse.bass as bass
    import concourse.tile as tile
    from concourse import bacc, mybir

    f32 = mybir.dt.float32
    bf16 = mybir.dt.bfloat16
    Exp = mybir.ActivationFunctionType.Exp
    Sigmoid = mybir.ActivationFunctionType.Sigmoid
    mult = mybir.AluOpType.mult
    add = mybir.AluOpType.add

    nc = bacc.Bacc("TRN2", target_bir_lowering=False, debug=False)
    xfull_d = nc.declare_dram_parameter("xfull", [KT, P, N], f32, isOutput=False)
    xn_d = nc.declare_dram_parameter("xn", [KT, P, NSLICE], f32, isOutput=False)
    wqT_d = nc.declare_dram_parameter("wqT", [KT, P, 3, R], f32, isOutput=False)
    wkT_d = nc.declare_dram_parameter("wkT", [KT, P, 3, R], f32, isOutput=False)
    wvT_d = nc.declare_dram_parameter("wvT", [KT, P, 3, R], f32, isOutput=False)
    woT_d = nc.declare_dram_parameter("woT", [3, R, C], f32, isOutput=False)
    y_d = nc.declare_dram_parameter("y", [KT, P, NSLICE], f32, isOutput=True)

    vt_width = 384 if VT_LAYOUT == "interleave" else 256

    with tile.TileContext(nc) as tc, ExitStack() as ctx:
        consts = ctx.enter_context(tc.tile_pool(name="consts", bufs=1))
        big_ps = ctx.enter_context(tc.tile_pool(name="big_ps", bufs=2, space="PSUM"))
        mm2_ps = ctx.enter_context(tc.tile_pool(name="mm2_ps", bufs=2, space="PSUM"))
        e_pool = ctx.enter_context(tc.tile_pool(name="e_pool", bufs=2))
        rb_pool = ctx.enter_context(tc.tile_pool(name="rb_pool", bufs=2))
        small = ctx.enter_context(tc.tile_pool(name="small", bufs=2))

        # ---- persistent SBUF state ----
        x_sb = consts.tile([P, KT, N], f32)
        nc.gpsimd.dma_start(x_sb[:], xfull_d.rearrange("t p m -> p t m"))
        xn_sb = consts.tile([P, KT, NSLICE], f32)
        nc.gpsimd.dma_start(xn_sb[:], xn_d.rearrange("t p m -> p t m"))
        wqT_sb = consts.tile([P, KT, 3, R], f32)
        nc.gpsimd.dma_start(wqT_sb[:], wqT_d.rearrange("t p i r -> p t i r"))
        wkT_sb = consts.tile([P, KT, 3, R], f32)
        nc.gpsimd.dma_start(wkT_sb[:], wkT_d.rearrange("t p i r -> p t i r"))
        wvT_sb = consts.tile([P, KT, 3, R], f32)
        nc.gpsimd.dma_start(wvT_sb[:], wvT_d.rearrange("t p i r -> p t i r"))

        woT_sb = []
        for i in range(3):
            w = consts.tile([P, C], f32, tag=f"woT{i}")
            nc.vector.memset(w[:], 0.0)
            nc.gpsimd.dma_start(w[0:R, :], woT_d[i])
            woT_sb.append(w)

        # q/k stored zero-padded to 128 partitions (contraction dim for mm1)
        q_sb = consts.tile([P, 3, NSLICE], bf16)
        nc.vector.memset(q_sb[:], 0.0)
        k_sb = consts.tile([P, 3, N], bf16)
        nc.vector.memset(k_sb[:], 0.0)

        # vT buffer: per m-tile, [vT_0 vT_1 vT_2 | ones64] (2dap) or
        # [vT_0 ones | vT_1 ones | vT_2 ones] (interleave)
        vT_buf = consts.tile([P, MT, vt_width], bf16)
        if VT_LAYOUT == "interleave":
            for j in range(3):
                nc.vector.memset(vT_buf[:, :, 128 * j + 64 : 128 * j + 128], 1.0)
        else:
            nc.vector.memset(vT_buf[:, :, 192:256], 1.0)

        # acc_i zero-padded to 128 partitions (contraction dim for final mm)
        acc = []
        for i in range(3):
            a = consts.tile([P, NSLICE], f32, tag=f"acc{i}")
            nc.vector.memset(a[:], 0.0)
            acc.append(a)

        def mm1_lhsT(j, mt):
            return k_sb[:, j, mt * P : (mt + 1) * P]

        def mm2_lhsT(j, mt):
            base = vT_buf[:, mt, :]
            if VT_LAYOUT == "interleave":
                return base[:, 128 * j : 128 * (j + 1)]
            # cols {64j .. 64j+63} U {192 .. 255} as a 2D free AP
            return bass.AP(
                tensor=base.tensor,
                offset=base.offset + 64 * j,
                ap=[base.ap[0], [192 - 64 * j, 2], [1, R]],
            )

        # ---- projections ----
        def emit_q(i):
            pt = big_ps.tile([P, NSLICE], f32, tag="big")
            for kt in range(KT):
                for c0, cw in CHUNKS:
                    nc.tensor.matmul(
                        pt[:R, c0 : c0 + cw],
                        wqT_sb[:, kt, i, :],
                        xn_sb[:, kt, c0 : c0 + cw],
                        start=(kt == 0),
                        stop=(kt == KT - 1),
                    )
            nc.vector.tensor_copy(q_sb[0:R, i, :], pt[0:R, :])

        def emit_k(j):
            for half in range(2):
                pt = big_ps.tile([P, NSLICE], f32, tag="big")
                for kt in range(KT):
                    for c0, cw in CHUNKS:
                        nc.tensor.matmul(
                            pt[:R, c0 : c0 + cw],
                            wkT_sb[:, kt, j, :],
                            x_sb[:, kt, half * NSLICE + c0 : half * NSLICE + c0 + cw],
                            start=(kt == 0),
                            stop=(kt == KT - 1),
                        )
                nc.vector.tensor_copy(
                    k_sb[0:R, j, half * NSLICE : (half + 1) * NSLICE], pt[0:R, :]
                )

        def emit_vT(j):
            col0 = 128 * j if VT_LAYOUT == "interleave" else 64 * j
            for mt in range(MT):
                pv = mm2_ps.tile([P, 512], f32, tag="mm2")
                for kt in range(KT):
                    nc.tensor.matmul(
                        pv[:, 0:R],
                        x_sb[:, kt, mt * P : (mt + 1) * P],
                        wvT_sb[:, kt, j, :],
                        start=(kt == 0),
                        stop=(kt == KT - 1),
                    )
                nc.vector.tensor_copy(vT_buf[:, mt, col0 : col0 + R], pv[:, 0:R])

        # ---- attention pair pipeline ----
        def emit_mm1_exp(i, j):
            E = e_pool.tile([P, MT, NSLICE], bf16, tag="E")
            for mt in range(MT):
                pt = big_ps.tile([P, NSLICE], f32, tag="big")
                for c0, cw in CHUNKS:
                    nc.tensor.matmul(
                        pt[:, c0 : c0 + cw],
                        mm1_lhsT(j, mt),
                        q_sb[:, i, c0 : c0 + cw],
                        start=True,
                        stop=True,
                    )
                nc.scalar.activation(E[:, mt, :], pt[:], Exp, scale=0.125)
            return E

        def emit_mm2_norm(i, j, E):
            for c0, cw in CHUNKS:
                pa = mm2_ps.tile([P, 512], f32, tag="mm2")
                for mt in range(MT):
                    nc.tensor.matmul(
                        pa[:, 0:cw],
                        mm2_lhsT(j, mt),
                        E[:, mt, c0 : c0 + cw],
                        start=(mt == 0),
                        stop=(mt == MT - 1),
                    )
                rb = rb_pool.tile([R, 512], f32, tag="rb")
                if NORM_MODE == "dmabcast":
                    zrow = pa[R : R + 1, 0:cw].partition_broadcast(R)
                    nc.sync.dma_start(rb[:, 0:cw], zrow.squeeze(1))
                else:
                    nc.vector.tensor_copy(rb[0:32, 0:cw], pa[64:96, 0:cw])
                    nc.vector.tensor_copy(rb[32:64, 0:cw], pa[96:128, 0:cw])
                nc.vector.reciprocal(rb[:, 0:cw], rb[:, 0:cw])
                if j == 0:
                    nc.vector.tensor_tensor(
                        acc[i][0:R, c0 : c0 + cw], pa[0:R, 0:cw], rb[:, 0:cw], mult
                    )
                else:
                    tmp = small.tile([R, 512], f32, tag="tmp")
                    nc.vector.tensor_tensor(
                        tmp[:, 0:cw], pa[0:R, 0:cw], rb[:, 0:cw], mult
                    )
                    nc.vector.tensor_tensor(
                        acc[i][0:R, c0 : c0 + cw],
                        acc[i][0:R, c0 : c0 + cw],
                        tmp[:, 0:cw],
                        add,
                    )

        for i in range(3):
            emit_q(i)

        prev = None
        for j in range(3):
            emit_k(j)
            emit_vT(j)
            for i in range(3):
                E = emit_mm1_exp(i, j)
                if prev is not None:
                    emit_mm2_norm(prev[0], prev[1], prev[2])
                prev = (i, j, E)
        emit_mm2_norm(prev[0], prev[1], prev[2])

        # ---- output projection + gating ----
        for mtile in range(KT):
            po = big_ps.tile([P, NSLICE], f32, tag="big")
            for c0, cw in CHUNKS:
                for i in range(3):
                    nc.tensor.matmul(
                        po[:, c0 : c0 + cw],
                        woT_sb[i][:, mtile * P : (mtile + 1) * P],
                        acc[i][:, c0 : c0 + cw],
                        start=(i == 0),
                        stop=(i == 2),
                    )
            sig = small.tile([P, NSLICE], f32, tag="sig")
            nc.scalar.activation(sig[:], po[:], Sigmoid)
            y_sb = small.tile([P, NSLICE], f32, tag="ysb")
            nc.vector.tensor_tensor(y_sb[:], xn_sb[:, mtile, :], sig[:], mult)
            nc.sync.dma_start(y_d[mtile], y_sb[:])

    nc.compile()
    return nc


def _get_program():
    if "nc" not in _CACHE:
        _CACHE["nc"] = _build_program()
    return _CACHE["nc"]


def _host_prep(x, wq, wk, wv, wo):
    xf = np.ascontiguousarray(x.reshape(B, C, N), dtype=np.float32)
    # wq: [3, R, C] -> wqT: [C, 3, R] -> [KT, P, 3, R]
    wqT = np.ascontiguousarray(
        np.transpose(wq, (2, 0, 1)).reshape(KT, P, 3, R), dtype=np.float32
    )
    wkT = np.ascontiguousarray(
        np.transpose(wk, (2, 0, 1)).reshape(KT, P, 3, R), dtype=np.float32
    )
    wvT = np.ascontiguousarray(
        np.transpose(wv, (2, 0, 1)).reshape(KT, P, 3, R), dtype=np.float32
    )
    # wo: [C, 3R] -> woT[i] = wo[:, 64i:64(i+1)].T
    woT = np.ascontiguousarray(
        np.stack([wo[:, R * i : R * (i + 1)].T for i in range(3)]), dtype=np.float32
    )
    in_maps = []
    for core in range(N_CORES):
        b, h = core // 2, core % 2
        xb = xf[b].reshape(KT, P, N)
        in_maps.append(
            {
                "xfull": xb,
                "xn": np.ascontiguousarray(xb[:, :, h * NSLICE : (h + 1) * NSLICE]),
                "wqT": wqT,
                "wkT": wkT,
                "wvT": wvT,
                "woT": woT,
            }
        )
    return in_maps


def kernel(x, wq, wk, wv, wo):
    global LAST_RESULTS
    from concourse.bass_utils import run_bass_kernel_spmd

    x = np.asarray(x)
    nc = _get_program()
    in_maps = _host_prep(
        x, np.asarray(wq), np.asarray(wk), np.asarray(wv), np.asarray(wo)
    )
    res = run_bass_kernel_spmd(nc, in_maps, core_ids=list(range(N_CORES)))
    LAST_RESULTS = res
    out = np.empty((B, C, N), np.float32)
    for core in range(N_CORES):
        b, h = core // 2, core % 2
        out[b][:, h * NSLICE : (h + 1) * NSLICE] = res.results[core]["y"].reshape(
            C, NSLICE
        )
    return out.reshape(B, C, H, W).astype(x.dtype, copy=False)


# revision 9
# speedup vs baseline: 1.3773x; 1.3773x over previous
"""NestedAttention Trainium2 kernel.

Reference computation (per batch b):
  q_i = wq[i] @ x ; k_j = wk[j] @ x ; v_j = wv[j] @ x        (1x1 convs, r=64)
  for i: acc_i = sum_j softmax_m(q_i^T k_j / sqrt(r)) applied to v_j
  out = wo @ concat_i(acc_i) ; y = x * sigmoid(out)

Sharding: 8 cores = batch(4) x query-column-halves(2). Each core holds full
k/v (m = 2304 keys) and a 1152-wide slice of query columns n; no cross-core
communication is needed (softmax is over m, fully on-core).

Per-core dataflow ([m, n] logit layout so the softmax sum over m comes out of
the second matmul for free):
  T_ij[m, n]   = k_j^T q_i           (PE, bf16 inputs, K=r padded to 128)
  E_ij         = exp(T * 0.125)      (ScalarE, PSUM->SBUF bf16)
  P_aug[128,n] = [vT_j | ones64]^T @ E_ij   rows 0:64 = v@P, rows 64:128 = Z
  acc_i[r, n] += P_aug[0:64] * (1/Z)
  out[c, n]    = sum_i woT_i^T @ acc_i ; y = xn * sigmoid(out)
"""

import os
import numpy as np

B, C, H, W = 4, 256, 48, 48
N = H * W            # 2304 keys (m) per image
NSLICE = N // 2      # 1152 query columns (n) per core
R = 64               # reduced channels
P = 128
MT = N // P          # 18 m-tiles
KT = C // P          # 2 contraction tiles over channels
CHUNKS = [(0, 512), (512, 512), (1024, 128)]  # n chunks, PSUM-bank aligned
N_CORES = 8

NORM_MODE = os.environ.get("NESTED_NORM_MODE", "xquad")   # xquad | dmabcast
VT_LAYOUT = os.environ.get("NESTED_VT_LAYOUT", "interleave")  # 2dap | interleave

_CACHE = {}
LAST_RESULTS = None


def _build_program():
    from contextlib import ExitStack

    import concourse.bass as bass
    import concourse.tile as tile
    from concourse import bacc, mybir

    f32 = mybir.dt.float32
    bf16 = mybir.dt.bfloat16
    Exp = mybir.ActivationFunctionType.Exp
    Sigmoid = mybir.ActivationFunctionType.Sigmoid
    mult = mybir.AluOpType.mult
    add = mybir.AluOpType.add

    nc = bacc.Bacc("TRN2", target_bir_lowering=False, debug=False)
    xb_d = nc.declare_dram_parameter("xb", [KT, P, N], bf16, isOutput=False)
    xnb_d = nc.declare_dram_parameter("xnb", [KT, P, NSLICE], bf16, isOutput=False)
    xn_d = nc.declare_dram_parameter("xn", [KT, P, NSLICE], f32, isOutput=False)
    wqT_d = nc.declare_dram_parameter("wqT", [KT, P, 3, R], bf16, isOutput=False)
    wkT_d = nc.declare_dram_parameter("wkT", [KT, P, 3, R], bf16, isOutput=False)
    wvT_d = nc.declare_dram_parameter("wvT", [KT, P, 3, R], bf16, isOutput=False)
    woT_d = nc.declare_dram_parameter("woT", [3, R, C], f32, isOutput=False)
    y_d = nc.declare_dram_parameter("y", [KT, P, NSLICE], f32, isOutput=True)

    vt_width = 384 if VT_LAYOUT == "interleave" else 256

    with tile.TileContext(nc) as tc, ExitStack() as ctx:
        consts = ctx.enter_context(tc.tile_pool(name="consts", bufs=1))
        big_ps = ctx.enter_context(tc.tile_pool(name="big_ps", bufs=2, space="PSUM"))
        mm2_ps = ctx.enter_context(tc.tile_pool(name="mm2_ps", bufs=2, space="PSUM"))
        e_pool = ctx.enter_context(tc.tile_pool(name="e_pool", bufs=2))
        rb_pool = ctx.enter_context(tc.tile_pool(name="rb_pool", bufs=2))
        small = ctx.enter_context(tc.tile_pool(name="small", bufs=2))

        # ---- persistent SBUF state ----
        x_sb = consts.tile([P, KT, N], bf16)
        nc.gpsimd.dma_start(x_sb[:], xb_d.rearrange("t p m -> p t m"))
        xnb_sb = consts.tile([P, KT, NSLICE], bf16)
        nc.gpsimd.dma_start(xnb_sb[:], xnb_d.rearrange("t p m -> p t m"))
        xn_sb = consts.tile([P, KT, NSLICE], f32)
        nc.gpsimd.dma_start(xn_sb[:], xn_d.rearrange("t p m -> p t m"))
        wqT_sb = consts.tile([P, KT, 3, R], bf16)
        nc.gpsimd.dma_start(wqT_sb[:], wqT_d.rearrange("t p i r -> p t i r"))
        wkT_sb = consts.tile([P, KT, 3, R], bf16)
        nc.gpsimd.dma_start(wkT_sb[:], wkT_d.rearrange("t p i r -> p t i r"))
        wvT_sb = consts.tile([P, KT, 3, R], bf16)
        nc.gpsimd.dma_start(wvT_sb[:], wvT_d.rearrange("t p i r -> p t i r"))

        woT_sb = []
        for i in range(3):
            w = consts.tile([P, C], f32, tag=f"woT{i}")
            nc.vector.memset(w[:], 0.0)
            nc.gpsimd.dma_start(w[0:R, :], woT_d[i])
            woT_sb.append(w)

        # q/k stored zero-padded to 128 partitions (contraction dim for mm1)
        q_sb = consts.tile([P, 3, NSLICE], bf16)
        nc.vector.memset(q_sb[:], 0.0)
        k_sb = consts.tile([P, 3, N], bf16)
        nc.vector.memset(k_sb[:], 0.0)

        # vT buffer: per m-tile, [vT_0 vT_1 vT_2 | ones64] (2dap) or
        # [vT_0 ones | vT_1 ones | vT_2 ones] (interleave)
        vT_buf = consts.tile([P, MT, vt_width], bf16)
        if VT_LAYOUT == "interleave":
            for j in range(3):
                nc.vector.memset(vT_buf[:, :, 128 * j + 64 : 128 * j + 128], 1.0)
        else:
            nc.vector.memset(vT_buf[:, :, 192:256], 1.0)

        # acc_i zero-padded to 128 partitions (contraction dim for final mm)
        acc = []
        for i in range(3):
            a = consts.tile([P, NSLICE], f32, tag=f"acc{i}")
            nc.vector.memset(a[:], 0.0)
            acc.append(a)

        def mm1_lhsT(j, mt):
            return k_sb[:, j, mt * P : (mt + 1) * P]

        def mm2_lhsT(j, mt):
            base = vT_buf[:, mt, :]
            if VT_LAYOUT == "interleave":
                return base[:, 128 * j : 128 * (j + 1)]
            # cols {64j .. 64j+63} U {192 .. 255} as a 2D free AP
            return bass.AP(
                tensor=base.tensor,
                offset=base.offset + 64 * j,
                ap=[base.ap[0], [192 - 64 * j, 2], [1, R]],
            )

        # ---- projections ----
        def emit_q(i):
            pt = big_ps.tile([P, NSLICE], f32, tag="big")
            for kt in range(KT):
                for c0, cw in CHUNKS:
                    nc.tensor.matmul(
                        pt[:R, c0 : c0 + cw],
                        wqT_sb[:, kt, i, :],
                        xnb_sb[:, kt, c0 : c0 + cw],
                        start=(kt == 0),
                        stop=(kt == KT - 1),
                    )
            nc.vector.tensor_copy(q_sb[0:R, i, :], pt[0:R, :])

        def emit_k(j):
            for half in range(2):
                pt = big_ps.tile([P, NSLICE], f32, tag="big")
                for kt in range(KT):
                    for c0, cw in CHUNKS:
                        nc.tensor.matmul(
                            pt[:R, c0 : c0 + cw],
                            wkT_sb[:, kt, j, :],
                            x_sb[:, kt, half * NSLICE + c0 : half * NSLICE + c0 + cw],
                            start=(kt == 0),
                            stop=(kt == KT - 1),
                        )
                nc.vector.tensor_copy(
                    k_sb[0:R, j, half * NSLICE : (half + 1) * NSLICE], pt[0:R, :]
                )

        def emit_vT_all():
            for mt in range(MT):
                pv = mm2_ps.tile([P, 512], f32, tag="mm2")
                for kt in range(KT):
                    nc.tensor.matmul(
                        pv[:, 0 : 3 * R],
                        x_sb[:, kt, mt * P : (mt + 1) * P],
                        wvT_sb[:, kt, :, :],
                        start=(kt == 0),
                        stop=(kt == KT - 1),
                    )
                base = vT_buf[:, mt, :]
                if VT_LAYOUT == "interleave":
                    dst = bass.AP(
                        tensor=base.tensor,
                        offset=base.offset,
                        ap=[base.ap[0], [128, 3], [1, R]],
                    )
                else:
                    dst = bass.AP(
                        tensor=base.tensor,
                        offset=base.offset,
                        ap=[base.ap[0], [R, 3], [1, R]],
                    )
                nc.vector.tensor_copy(
                    dst, pv[:, 0 : 3 * R].rearrange("p (j r) -> p j r", j=3)
                )

        # ---- attention pair pipeline ----
        def emit_mm1_exp(i, j):
            E = e_pool.tile([P, MT, NSLICE], bf16, tag="E")
            for mt in range(MT):
                pt = big_ps.tile([P, NSLICE], f32, tag="big")
                for c0, cw in CHUNKS:
                    nc.tensor.matmul(
                        pt[:, c0 : c0 + cw],
                        mm1_lhsT(j, mt),
                        q_sb[:, i, c0 : c0 + cw],
                        start=True,
                        stop=True,
                    )
                nc.scalar.activation(E[:, mt, :], pt[:], Exp, scale=0.125)
            return E

        def emit_mm2_norm(i, j, E):
            for c0, cw in CHUNKS:
                pa = mm2_ps.tile([P, 512], f32, tag="mm2")
                for mt in range(MT):
                    nc.tensor.matmul(
                        pa[:, 0:cw],
                        mm2_lhsT(j, mt),
                        E[:, mt, c0 : c0 + cw],
                        start=(mt == 0),
                        stop=(mt == MT - 1),
                    )
                rb = rb_pool.tile([R, 512], f32, tag="rb")
                if NORM_MODE == "dmabcast":
                    zrow = pa[R : R + 1, 0:cw].partition_broadcast(R)
                    nc.sync.dma_start(rb[:, 0:cw], zrow.squeeze(1))
                else:
                    nc.vector.tensor_copy(rb[0:32, 0:cw], pa[64:96, 0:cw])
                    nc.vector.tensor_copy(rb[32:64, 0:cw], pa[96:128, 0:cw])
                nc.vector.reciprocal_approx_fast(rb[:, 0:cw], rb[:, 0:cw])
                if j == 0:
                    nc.vector.tensor_tensor(
                        acc[i][0:R, c0 : c0 + cw], pa[0:R, 0:cw], rb[:, 0:cw], mult
                    )
                else:
                    tmp = small.tile([R, 512], f32, tag="tmp")
                    nc.vector.tensor_tensor(
                        tmp[:, 0:cw], pa[0:R, 0:cw], rb[:, 0:cw], mult
                    )
                    nc.vector.tensor_tensor(
                        acc[i][0:R, c0 : c0 + cw],
                        acc[i][0:R, c0 : c0 + cw],
                        tmp[:, 0:cw],
                        add,
                    )

        for i in range(3):
            emit_q(i)

        emit_vT_all()
        prev = None
        for j in range(3):
            emit_k(j)
            for i in range(3):
                E = emit_mm1_exp(i, j)
                if prev is not None:
                    emit_mm2_norm(prev[0], prev[1], prev[2])
                prev = (i, j, E)
        emit_mm2_norm(prev[0], prev[1], prev[2])

        # ---- output projection + gating ----
        for mtile in range(KT):
            po = big_ps.tile([P, NSLICE], f32, tag="big")
            for c0, cw in CHUNKS:
                for i in range(3):
                    nc.tensor.matmul(
                        po[:, c0 : c0 + cw],
                        woT_sb[i][:, mtile * P : (mtile + 1) * P],
                        acc[i][:, c0 : c0 + cw],
                        start=(i == 0),
                        stop=(i == 2),
                    )
            sig = small.tile([P, NSLICE], f32, tag="sig")
            nc.scalar.activation(sig[:], po[:], Sigmoid)
            y_sb = small.tile([P, NSLICE], f32, tag="ysb")
            nc.vector.tensor_tensor(y_sb[:], xn_sb[:, mtile, :], sig[:], mult)
            nc.sync.dma_start(y_d[mtile], y_sb[:])

    nc.compile()
    return nc


def _get_program():
    if "nc" not in _CACHE:
        _CACHE["nc"] = _build_program()
    return _CACHE["nc"]


def _host_prep(x, wq, wk, wv, wo):
    import ml_dtypes

    bf16 = ml_dtypes.bfloat16
    xf = np.ascontiguousarray(x.reshape(B, C, N), dtype=np.float32)
    # wq: [3, R, C] -> wqT: [C, 3, R] -> [KT, P, 3, R]
    wqT = np.ascontiguousarray(np.transpose(wq, (2, 0, 1)).reshape(KT, P, 3, R)).astype(bf16)
    wkT = np.ascontiguousarray(np.transpose(wk, (2, 0, 1)).reshape(KT, P, 3, R)).astype(bf16)
    wvT = np.ascontiguousarray(np.transpose(wv, (2, 0, 1)).reshape(KT, P, 3, R)).astype(bf16)
    # wo: [C, 3R] -> woT[i] = wo[:, 64i:64(i+1)].T
    woT = np.ascontiguousarray(
        np.stack([wo[:, R * i : R * (i + 1)].T for i in range(3)]), dtype=np.float32
    )
    in_maps = []
    for core in range(N_CORES):
        b, h = core // 2, core % 2
        xcore = xf[b].reshape(KT, P, N)
        xn32 = np.ascontiguousarray(xcore[:, :, h * NSLICE : (h + 1) * NSLICE])
        in_maps.append(
            {
                "xb": xcore.astype(bf16),
                "xnb": xn32.astype(bf16),
                "xn": xn32,
                "wqT": wqT,
                "wkT": wkT,
                "wvT": wvT,
                "woT": woT,
            }
        )
    return in_maps


def kernel(x, wq, wk, wv, wo):
    global LAST_RESULTS
    from concourse.bass_utils import run_bass_kernel_spmd

    x = np.asarray(x)
    nc = _get_program()
    in_maps = _host_prep(
        x, np.asarray(wq), np.asarray(wk), np.asarray(wv), np.asarray(wo)
    )
    res = run_bass_kernel_spmd(nc, in_maps, core_ids=list(range(N_CORES)))
    LAST_RESULTS = res
    out = np.empty((B, C, N), np.float32)
    for core in range(N_CORES):
        b, h = core // 2, core % 2
        out[b][:, h * NSLICE : (h + 1) * NSLICE] = res.results[core]["y"].reshape(
            C, NSLICE
        )
    return out.reshape(B, C, H, W).astype(x.dtype, copy=False)
